# revision 1
# baseline (speedup 1.0000x reference)
"""Trainium2 Bass kernel for sparse_attention scoring + softmax.

Computes, for full inputs:
    enc = encoder_outputs[0]                      # [S=32768, H=1024]
    energies = (enc @ W^T + b) @ hidden           # [S]
    attn = softmax(energies)                      # -> [1, 1, S]

Algebraic restructure: energies = enc @ (W^T @ hidden) + (b . hidden).
The additive constant (b . hidden) is dropped because softmax is invariant
to constant shifts.  The tiny [H] vector v = W^T @ hidden is computed on
host (0.003% of FLOPs); the device streams enc (sequence-parallel across
8 cores), computes per-shard energies with fused DVE multiply-reduce ops,
all-gathers the [S] energies and does the softmax on device.
"""

import sys

sys.path.insert(0, "/opt/trn_rl_repo")

from contextlib import ExitStack

import numpy as np

import concourse.bass as bass
import concourse.bacc as bacc
import concourse.mybir as mybir
import concourse.tile as tile
from concourse.bass_utils import run_bass_kernel_spmd

N_CORES = 8
SEQ = 32768
HID = 1024
SHARD = SEQ // N_CORES  # 4096 seq positions per core

# Main-loop tiling: outer DMA tiles of [128, K*HID] (K seq-row-groups per
# partition slot), processed as K fused multiply-reduce ops of [128, HID]
# each.  The first tiles are small so the DVE starts as soon as possible;
# later tiles are 2 MiB for DMA efficiency.
K_MAX = 4
ENC_BUFS = 6


def tile_schedule(n_col):
    """List of K values (in 128-row units) summing to n_col."""
    ks = []
    ramp = [1, 1, 2]
    for k in ramp:
        if sum(ks) + k <= n_col:
            ks.append(k)
    while sum(ks) < n_col:
        ks.append(min(K_MAX, n_col - sum(ks)))
    return ks


def build_body(nc, tc, enc, vb, ident, ones, out, n_cores=N_CORES, seq=SEQ,
               shard=SHARD):
    f32 = mybir.dt.float32
    mx = mybir.AluOpType.max
    n_col = shard // 128            # energy columns accumulated in SBUF
    seq_f = seq // 128              # free size of the full-seq softmax tile

    ctx = ExitStack()
    cpool = ctx.enter_context(tc.tile_pool(name="cpool", bufs=1))
    iopool = ctx.enter_context(tc.tile_pool(name="iopool", bufs=ENC_BUFS))
    wpool = ctx.enter_context(tc.tile_pool(name="wpool", bufs=2))
    dpool = ctx.enter_context(tc.tile_pool(name="dpool", bufs=1, space="DRAM"))
    pspool = ctx.enter_context(tc.tile_pool(name="pspool", bufs=1, space="PSUM"))

    # --- setup: v (pre-broadcast on host) — emitted FIRST so its DMA and the
    # first enc tile's DMA hit the queues before anything else.
    v_sb = cpool.tile([128, HID], f32)
    nc.sync.dma_start(out=v_sb[:, :], in_=vb[:, :])

    # Early throwaway exp so the ~2.4us ACT_TABLE_LOAD(+drain) runs during
    # the main loop; without it the table load lands on the softmax
    # critical path right before the real exp (seen in every trace).
    warm = wpool.tile([1, 1], f32, tag="warm")
    nc.scalar.activation(
        out=warm[:, :], in_=v_sb[0:1, 0:1],
        func=mybir.ActivationFunctionType.Exp,
        bias=v_sb[0:1, 0:1],
    )

    # Warm-up collective: a tiny AllGather issued up front (hidden under the
    # main loop) so the real one hits a warm ncfw/comm path.  Reads the
    # ident DRAM input directly (no SBUF dependency).
    warm_in = dpool.tile([8], f32)
    warm_out = dpool.tile([8 * n_cores], f32, addr_space="Shared")
    nc.sync.dma_start(out=warm_in.rearrange("(a b) -> a b", a=1),
                      in_=ident[0:1, 0:8])
    nc.gpsimd.collective_compute(
        "AllGather",
        mybir.AluOpType.bypass,
        replica_groups=[list(range(n_cores))],
        ins=[warm_in.opt()],
        outs=[warm_out.opt()],
    )

    # --- main loop: energies[p, j] for shard-local seq = j*128 + p ---
    # The energy transpose + e_loc push happens in two halves: the first
    # half right after column n_col/2 completes (hidden under the loop),
    # only the second half sits on the post-loop critical path.
    e_sb = cpool.tile([128, n_col], f32)
    enc_r = enc.rearrange("(j p) h -> p j h", p=128)   # [128, n_col, HID]

    ident_sb = cpool.tile([128, 128], f32)
    ones_sb = cpool.tile([128, 128], f32)
    e_loc = dpool.tile([shard], f32)
    e_loc_r = e_loc.rearrange("(j p) -> j p", p=128)
    e_all = dpool.tile([seq], f32, addr_space="Shared")
    half = n_col // 2

    def push_energies(lo, hi):
        et_ps = pspool.tile([hi - lo, 128], f32, tag="et", name=f"et_ps_{lo}")
        nc.tensor.transpose(et_ps[:, :], e_sb[:, lo:hi], ident_sb[:, :])
        et_sb = cpool.tile([hi - lo, 128], f32, name=f"et_sb_{lo}")
        nc.vector.tensor_copy(et_sb[:, :], et_ps[:, :])
        nc.sync.dma_start(out=e_loc_r[lo:hi, :], in_=et_sb[:, :])

    j0 = 0
    for t, kt in enumerate(tile_schedule(n_col)):
        buf = iopool.tile([128, K_MAX * HID], f32, tag="enc")
        bufv = buf.rearrange("p (k h) -> p k h", k=K_MAX)
        nc.sync.dma_start(out=bufv[:, 0:kt, :], in_=enc_r[:, j0:j0 + kt, :])
        scratch = wpool.tile([128, HID], f32, tag="scratch")
        for k in range(kt):
            j = j0 + k
            # fused multiply + free-dim-sum: out = (in0 * 1.0) * v,
            # accum_out = sum(out).  (tensor_tensor_reduce crashes trn2 HW
            # under this compile path; scalar_tensor_tensor is equivalent.)
            nc.vector.scalar_tensor_tensor(
                out=scratch[:, :],
                in0=buf[:, k * HID:(k + 1) * HID],
                scalar=1.0,
                in1=v_sb[:, :],
                op0=mybir.AluOpType.mult,
                op1=mybir.AluOpType.mult,
                accum_out=e_sb[:, j:j + 1],
            )
        j0 += kt
        if j0 == half:
            # consts needed by push_energies and the softmax tail; emitted
            # here so their DMAs don't compete with the first enc tiles.
            nc.sync.dma_start(out=ident_sb[:, :], in_=ident[:, :])
            nc.sync.dma_start(out=ones_sb[:, :], in_=ones[:, :])
            push_energies(0, half)

    ones_row = ones_sb[0:1, :]
    ones_col = ones_sb[:, 0:1]
    push_energies(half, n_col)

    nc.gpsimd.collective_compute(
        "AllGather",
        mybir.AluOpType.bypass,
        replica_groups=[list(range(n_cores))],
        ins=[e_loc.opt()],
        outs=[e_all.opt()],
    )

    # --- softmax over the full gathered energies ---
    es = iopool.tile([128, seq_f], f32, tag="es")
    nc.sync.dma_start(out=es[:, :], in_=e_all.rearrange("(p f) -> p f", p=128))

    # per-partition max, then cross-partition max via PE transpose + reduce
    m1 = wpool.tile([128, 1], f32, tag="m1", bufs=1)
    nc.vector.tensor_reduce(
        out=m1[:, :], in_=es[:, :], axis=mybir.AxisListType.X, op=mx,
    )
    m1t_ps = pspool.tile([1, 128], f32, tag="m1t")
    nc.tensor.matmul(m1t_ps[:, :], m1[:, :], ident_sb[:, :],
                     start=True, stop=True)
    gmx = wpool.tile([1, 1], f32, tag="gmx", bufs=1)
    nc.vector.tensor_reduce(
        out=gmx[:, :], in_=m1t_ps[:, :], axis=mybir.AxisListType.X, op=mx,
    )
    # broadcast global max to [128,1] via PE ones-row matmul.
    gm_ps = pspool.tile([128, 1], f32, tag="gm")
    nc.tensor.matmul(gm_ps[:, :], ones_row, gmx[0:1, 0:1],
                     start=True, stop=True)
    ngm = wpool.tile([128, 1], f32, tag="ngm", bufs=1)
    nc.vector.tensor_scalar_mul(ngm[:, :], gm_ps[:, :], -1.0)

    a = iopool.tile([128, seq_f], f32, tag="a")
    ssum = wpool.tile([128, 1], f32, tag="ssum", bufs=1)
    nc.scalar.activation(
        out=a[:, :], in_=es[:, :],
        func=mybir.ActivationFunctionType.Exp,
        bias=ngm[:, :], scale=1.0,
        accum_out=ssum[:, :],
    )
    # global sum: ssum.T @ ones -> [1,1]; reciprocal; broadcast back.
    s_ps = pspool.tile([1, 1], f32, tag="s")
    nc.tensor.matmul(s_ps[:, :], ssum[:, :], ones_col,
                     start=True, stop=True)
    rs = wpool.tile([1, 1], f32, tag="rs", bufs=1)
    nc.vector.reciprocal(rs[:, :], s_ps[:, :])
    r_ps = pspool.tile([128, 1], f32, tag="r")
    nc.tensor.matmul(r_ps[:, :], ones_row, rs[0:1, 0:1],
                     start=True, stop=True)
    r_sb = wpool.tile([128, 1], f32, tag="rsb", bufs=1)
    nc.vector.tensor_copy(r_sb[:, :], r_ps[:, :])

    a2 = iopool.tile([128, seq_f], f32, tag="a2")
    nc.vector.tensor_scalar_mul(a2[:, :], a[:, :], r_sb[:, :])
    nc.sync.dma_start(out=out.rearrange("(p f) -> p f", p=128), in_=a2[:, :])

    ctx.close()


def build_nc(n_cores=N_CORES, seq=SEQ, shard=SHARD, debug=False):
    nc = bacc.Bacc(
        "TRN2",
        target_bir_lowering=False,
        debug=debug,
        num_devices=n_cores,
    )
    enc = nc.dram_tensor("enc", [shard, HID], mybir.dt.float32, kind="ExternalInput")
    vb = nc.dram_tensor("vb", [128, HID], mybir.dt.float32, kind="ExternalInput")
    ident = nc.dram_tensor("ident", [128, 128], mybir.dt.float32, kind="ExternalInput")
    ones = nc.dram_tensor("ones", [128, 128], mybir.dt.float32, kind="ExternalInput")
    out = nc.dram_tensor("attn", [seq], mybir.dt.float32, kind="ExternalOutput")
    with tile.TileContext(nc) as tc:
        build_body(nc, tc, enc.ap(), vb.ap(), ident.ap(), ones.ap(), out.ap(),
                   n_cores=n_cores, seq=seq, shard=shard)
    nc.compile()
    return nc


_NC_CACHE = {}


def _get_nc():
    if "nc" not in _NC_CACHE:
        _NC_CACHE["nc"] = build_nc()
    return _NC_CACHE["nc"]


def make_in_maps(hidden, encoder_outputs, attn_w, attn_b=None, n_cores=N_CORES,
                 shard=SHARD):
    hidden = np.asarray(hidden, dtype=np.float32)
    enc = np.asarray(encoder_outputs, dtype=np.float32)[0]
    w = np.asarray(attn_w, dtype=np.float32)
    v = (w.T @ hidden).astype(np.float32)
    vb = np.ascontiguousarray(np.broadcast_to(v[None, :], (128, v.shape[0])))
    ident = np.eye(128, dtype=np.float32)
    ones = np.ones((128, 128), dtype=np.float32)
    return [
        {
            "enc": np.ascontiguousarray(enc[i * shard:(i + 1) * shard, :]),
            "vb": vb,
            "ident": ident,
            "ones": ones,
        }
        for i in range(n_cores)
    ]


def run(in_maps, trace=False, **kwargs):
    nc = _get_nc()
    return run_bass_kernel_spmd(
        nc, in_maps, core_ids=list(range(N_CORES)), trace=trace, **kwargs
    )


def kernel(**inputs):
    in_maps = make_in_maps(
        inputs["hidden"], inputs["encoder_outputs"], inputs["attn_w"],
        inputs.get("attn_b"),
    )
    res = run(in_maps)
    attn = np.asarray(res.results[0]["attn"], dtype=np.float32).reshape(-1)
    return attn[None, None, :]



# revision 2
# speedup vs baseline: 1.0144x; 1.0144x over previous
"""Trainium2 Bass kernel for sparse_attention scoring + softmax.

Computes, for full inputs:
    enc = encoder_outputs[0]                      # [S=32768, H=1024]
    energies = (enc @ W^T + b) @ hidden           # [S]
    attn = softmax(energies)                      # -> [1, 1, S]

Algebraic restructure: energies = enc @ (W^T @ hidden) + (b . hidden).
The additive constant (b . hidden) is dropped because softmax is invariant
to constant shifts.  The tiny [H] vector v = W^T @ hidden is computed on
host (0.003% of FLOPs); each of the 8 cores streams its seq-shard of enc
(partition-major, so every DMA line is contiguous), computes per-shard
energies with fused DVE multiply-reduce ops, then does a distributed
softmax: exp with the LOCAL max, then an 8-byte-per-core AllGather of
(local_max, local_expsum) pairs, and a final per-shard rescale by
exp(m_i - M) / S -- mathematically the exact global softmax.  Each core
outputs only its own [4096] shard; the host concatenates.
"""

import sys

sys.path.insert(0, "/opt/trn_rl_repo")

from contextlib import ExitStack

import numpy as np

import concourse.bass as bass
import concourse.bacc as bacc
import concourse.mybir as mybir
import concourse.tile as tile
from concourse.bass_utils import run_bass_kernel_spmd

N_CORES = 8
SEQ = 32768
HID = 1024
SHARD = SEQ // N_CORES      # 4096 seq positions per core
N_COL = SHARD // 128        # 32 energy columns per core

# Outer DMA tiles of [128, K*HID] (K seq rows per partition), processed as
# K fused multiply-reduce ops of [128, HID] each.  Small first tiles so the
# DVE starts ASAP; small last tiles so the post-loop DVE tail is short.
K_MAX = 4
ENC_BUFS = 6
SCHEDULE = [1, 1, 2, 4, 4, 4, 4, 4, 4, 2, 1, 1]     # sums to N_COL = 32
assert sum(SCHEDULE) == N_COL


def build_body(nc, tc, enc, vb, ident, out, n_cores=N_CORES):
    f32 = mybir.dt.float32
    mx = mybir.AluOpType.max
    mult = mybir.AluOpType.mult
    groups = [list(range(n_cores))]

    ctx = ExitStack()
    cpool = ctx.enter_context(tc.tile_pool(name="cpool", bufs=1))
    iopool = ctx.enter_context(tc.tile_pool(name="iopool", bufs=ENC_BUFS))
    wpool = ctx.enter_context(tc.tile_pool(name="wpool", bufs=2))
    dwpool = ctx.enter_context(tc.tile_pool(name="dwpool", bufs=1, space="DRAM"))
    dgpool = ctx.enter_context(tc.tile_pool(name="dgpool", bufs=1, space="DRAM"))
    pspool = ctx.enter_context(tc.tile_pool(name="pspool", bufs=1, space="PSUM"))

    # --- first DMAs: v (sync ring) and the warmup-collective input (scalar
    # ring) are issued before anything else.
    v_sb = cpool.tile([128, HID], f32)
    nc.sync.dma_start(out=v_sb[:, :], in_=vb[:, :])

    # Warm-up collective: a tiny AllGather issued up front so the ncfw
    # barrier + first-collective software setup runs during the main loop
    # instead of in front of the real (m, s) gather.
    warm_in = dwpool.tile([8], f32)
    warm_out = dwpool.tile([8 * n_cores], f32, addr_space="Shared")
    nc.scalar.dma_start(out=warm_in.rearrange("(a b) -> a b", a=1),
                        in_=ident[0:1, 0:8])
    nc.gpsimd.collective_compute(
        "AllGather",
        mybir.AluOpType.bypass,
        replica_groups=groups,
        ins=[warm_in.opt()],
        outs=[warm_out.opt()],
    )

    # ones matrix generated on-chip (gpsimd is idle early); used for the
    # cross-partition sum and scalar broadcasts in the tail.
    ones_sb = cpool.tile([128, 128], f32)
    nc.gpsimd.memset(ones_sb[:, :], 1.0)

    # Early throwaway exp so the ~2.4us ACT_TABLE_LOAD(+drain) runs during
    # the main loop; without it the table load lands on the softmax
    # critical path right before the real exp.
    warm = wpool.tile([1, 1], f32, tag="warm")
    nc.scalar.activation(
        out=warm[:, :], in_=v_sb[0:1, 0:1],
        func=mybir.ActivationFunctionType.Exp,
        bias=v_sb[0:1, 0:1],
    )

    # --- main loop: e_sb[p, j] = energy of shard-local seq = p*N_COL + j ---
    # Partition-major layout: each partition's DMA line is one contiguous
    # kt*4KiB chunk of HBM, and the final attn store is contiguous too.
    e_sb = cpool.tile([128, N_COL], f32)
    enc_r = enc.rearrange("(p j) h -> p j h", p=128)   # [128, N_COL, HID]

    ident_sb = cpool.tile([128, 128], f32)

    j0 = 0
    for t, kt in enumerate(SCHEDULE):
        buf = iopool.tile([128, K_MAX * HID], f32, tag="enc")
        bufv = buf.rearrange("p (k h) -> p k h", k=K_MAX)
        # Alternate the two HWDGE rings (sync / scalar) so descriptor
        # generation and queue issue overlap across consecutive tiles.
        eng = nc.sync if t % 2 == 0 else nc.scalar
        eng.dma_start(out=bufv[:, 0:kt, :], in_=enc_r[:, j0:j0 + kt, :])
        scratch = wpool.tile([128, HID], f32, tag="scratch")
        for k in range(kt):
            j = j0 + k
            # fused multiply + free-dim-sum: out = (in0 * 1.0) * v,
            # accum_out = sum(out).  (tensor_tensor_reduce crashes trn2 HW
            # under this compile path; scalar_tensor_tensor is equivalent.)
            nc.vector.scalar_tensor_tensor(
                out=scratch[:, :],
                in0=buf[:, k * HID:(k + 1) * HID],
                scalar=1.0,
                in1=v_sb[:, :],
                op0=mult,
                op1=mult,
                accum_out=e_sb[:, j:j + 1],
            )
        j0 += kt
        if t == 3:
            # needed only in the tail; DMA'd mid-loop so it doesn't compete
            # with the first enc tiles.
            nc.scalar.dma_start(out=ident_sb[:, :], in_=ident[:, :])

    # --- tail: local softmax with local max ---
    m1 = wpool.tile([128, 1], f32, tag="m1", bufs=1)
    nc.vector.tensor_reduce(
        out=m1[:, :], in_=e_sb[:, :], axis=mybir.AxisListType.X, op=mx,
    )
    m1t_ps = pspool.tile([1, 128], f32, tag="m1t")
    nc.tensor.transpose(m1t_ps[:, :], m1[:, :], ident_sb[:, :])
    gmx = wpool.tile([1, 1], f32, tag="gmx", bufs=1)
    nc.vector.tensor_reduce(
        out=gmx[:, :], in_=m1t_ps[:, :], axis=mybir.AxisListType.X, op=mx,
    )
    ngm1 = wpool.tile([1, 1], f32, tag="ngm1", bufs=1)
    nc.vector.tensor_scalar_mul(ngm1[:, :], gmx[:, :], -1.0)
    nm_ps = pspool.tile([128, 1], f32, tag="nm")
    nc.tensor.matmul(nm_ps[:, :], ones_sb[0:1, :], ngm1[0:1, 0:1],
                     start=True, stop=True)
    nm_sb = wpool.tile([128, 1], f32, tag="nm_sb", bufs=1)
    nc.vector.tensor_copy(nm_sb[:, :], nm_ps[:, :])

    a_loc = cpool.tile([128, N_COL], f32)
    ssum = wpool.tile([128, 1], f32, tag="ssum", bufs=1)
    nc.scalar.activation(
        out=a_loc[:, :], in_=e_sb[:, :],
        func=mybir.ActivationFunctionType.Exp,
        bias=nm_sb[:, :], scale=1.0,
        accum_out=ssum[:, :],
    )
    s_ps = pspool.tile([1, 1], f32, tag="s")
    nc.tensor.matmul(s_ps[:, :], ssum[:, :], ones_sb[:, 0:1],
                     start=True, stop=True)

    # --- gather (m_i, s_i) from all cores: 8 bytes per core ---
    pk = wpool.tile([1, 2], f32, tag="pk", bufs=1)
    nc.vector.tensor_copy(pk[0:1, 0:1], gmx[0:1, 0:1])
    nc.vector.tensor_copy(pk[0:1, 1:2], s_ps[0:1, 0:1])
    ga_in = dgpool.tile([2], f32)
    ga_out = dgpool.tile([2 * n_cores], f32, addr_space="Shared")
    nc.sync.dma_start(out=ga_in.rearrange("(a b) -> a b", a=1),
                      in_=pk[0:1, 0:2])
    nc.gpsimd.collective_compute(
        "AllGather",
        mybir.AluOpType.bypass,
        replica_groups=groups,
        ins=[ga_in.opt()],
        outs=[ga_out.opt()],
    )
    g_r = ga_out.rearrange("(c k) -> k c", k=2)        # row 0: m's, row 1: s's
    mrow = wpool.tile([1, 8], f32, tag="mrow", bufs=1)
    srow = wpool.tile([1, 8], f32, tag="srow", bufs=1)
    nc.sync.dma_start(out=mrow[0:1, :], in_=g_r[0:1, :])
    nc.scalar.dma_start(out=srow[0:1, :], in_=g_r[1:2, :])

    # M = max_j m_j;  S = sum_j s_j * exp(m_j - M);  c = exp(m_i - M) / S
    M = wpool.tile([1, 1], f32, tag="M", bufs=1)
    nc.vector.tensor_reduce(
        out=M[:, :], in_=mrow[:, :], axis=mybir.AxisListType.X, op=mx,
    )
    nM = wpool.tile([1, 1], f32, tag="nM", bufs=1)
    nc.vector.tensor_scalar_mul(nM[:, :], M[:, :], -1.0)
    trow = wpool.tile([1, 8], f32, tag="trow", bufs=1)
    nc.scalar.activation(
        out=trow[:, :], in_=mrow[:, :],
        func=mybir.ActivationFunctionType.Exp,
        bias=nM[0:1, 0:1],
    )
    junk = wpool.tile([1, 8], f32, tag="junk", bufs=1)
    S = wpool.tile([1, 1], f32, tag="S", bufs=1)
    nc.vector.scalar_tensor_tensor(
        out=junk[:, :], in0=trow[:, :], scalar=1.0, in1=srow[:, :],
        op0=mult, op1=mult, accum_out=S[:, :],
    )
    rS = wpool.tile([1, 1], f32, tag="rS", bufs=1)
    nc.vector.reciprocal(rS[:, :], S[:, :])
    ee = wpool.tile([1, 1], f32, tag="ee", bufs=1)
    nc.scalar.activation(
        out=ee[:, :], in_=gmx[:, :],
        func=mybir.ActivationFunctionType.Exp,
        bias=nM[0:1, 0:1],
    )
    cfac = wpool.tile([1, 1], f32, tag="cfac", bufs=1)
    nc.vector.tensor_mul(cfac[:, :], ee[:, :], rS[:, :])
    c_ps = pspool.tile([128, 1], f32, tag="c")
    nc.tensor.matmul(c_ps[:, :], ones_sb[0:1, :], cfac[0:1, 0:1],
                     start=True, stop=True)
    c_sb = wpool.tile([128, 1], f32, tag="c_sb", bufs=1)
    nc.vector.tensor_copy(c_sb[:, :], c_ps[:, :])

    attn = cpool.tile([128, N_COL], f32)
    nc.vector.tensor_scalar_mul(attn[:, :], a_loc[:, :], c_sb[:, :])
    nc.sync.dma_start(out=out.rearrange("(p j) -> p j", p=128),
                      in_=attn[:, :])

    ctx.close()


def build_nc(n_cores=N_CORES, debug=False):
    nc = bacc.Bacc(
        "TRN2",
        target_bir_lowering=False,
        debug=debug,
        num_devices=n_cores,
    )
    enc = nc.dram_tensor("enc", [SHARD, HID], mybir.dt.float32, kind="ExternalInput")
    vb = nc.dram_tensor("vb", [128, HID], mybir.dt.float32, kind="ExternalInput")
    ident = nc.dram_tensor("ident", [128, 128], mybir.dt.float32, kind="ExternalInput")
    out = nc.dram_tensor("attn", [SHARD], mybir.dt.float32, kind="ExternalOutput")
    with tile.TileContext(nc) as tc:
        build_body(nc, tc, enc.ap(), vb.ap(), ident.ap(), out.ap(),
                   n_cores=n_cores)
    nc.compile()
    return nc


_NC_CACHE = {}


def _get_nc():
    if "nc" not in _NC_CACHE:
        _NC_CACHE["nc"] = build_nc()
    return _NC_CACHE["nc"]


def make_in_maps(hidden, encoder_outputs, attn_w, attn_b=None, n_cores=N_CORES,
                 shard=SHARD):
    hidden = np.asarray(hidden, dtype=np.float32)
    enc = np.asarray(encoder_outputs, dtype=np.float32)[0]
    w = np.asarray(attn_w, dtype=np.float32)
    v = (w.T @ hidden).astype(np.float32)
    vb = np.ascontiguousarray(np.broadcast_to(v[None, :], (128, v.shape[0])))
    ident = np.eye(128, dtype=np.float32)
    return [
        {
            "enc": np.ascontiguousarray(enc[i * shard:(i + 1) * shard, :]),
            "vb": vb,
            "ident": ident,
        }
        for i in range(n_cores)
    ]


def run(in_maps, trace=False, **kwargs):
    nc = _get_nc()
    return run_bass_kernel_spmd(
        nc, in_maps, core_ids=list(range(N_CORES)), trace=trace, **kwargs
    )


def kernel(**inputs):
    in_maps = make_in_maps(
        inputs["hidden"], inputs["encoder_outputs"], inputs["attn_w"],
        inputs.get("attn_b"),
    )
    res = run(in_maps)
    shards = [
        np.asarray(res.results[i]["attn"], dtype=np.float32).reshape(-1)
        for i in range(N_CORES)
    ]
    attn = np.concatenate(shards)
    return attn[None, None, :]


# revision 11
# speedup vs baseline: 1.4399x; 1.4194x over previous
"""Trainium2 Bass kernel for sparse_attention scoring + softmax.

Computes, for full inputs:
    enc = encoder_outputs[0]                      # [S=32768, H=1024]
    energies = (enc @ W^T + b) @ hidden           # [S]
    attn = softmax(energies)                      # -> [1, 1, S]

Algebraic restructure: energies = enc @ (W^T @ hidden) + (b . hidden).
The additive constant (b . hidden) is dropped because softmax is invariant
to constant shifts.  The tiny [H] vector v = W^T @ hidden is computed on
host (0.003% of FLOPs); each of the 8 cores streams its seq-shard of enc
(partition-major, so every DMA line is contiguous), computes per-shard
energies with fused DVE multiply-reduce ops, then does a distributed
softmax: exp with the LOCAL max, then an 8-byte-per-core AllGather of
(local_max, local_expsum) pairs, and a final per-shard rescale by
exp(m_i - M) / S -- mathematically the exact global softmax.  Each core
outputs only its own [4096] shard; the host concatenates.
"""

import sys

sys.path.insert(0, "/opt/trn_rl_repo")

from contextlib import ExitStack

import numpy as np

import concourse.bass as bass
import concourse.bacc as bacc
import concourse.mybir as mybir
import concourse.tile as tile
from concourse.bass_utils import run_bass_kernel_spmd

N_CORES = 8
SEQ = 32768
HID = 1024
SHARD = SEQ // N_CORES      # 4096 seq positions per core
N_COL = SHARD // 128        # 32 energy columns per core

# Outer DMA tiles of [128, K*HID] (K seq rows per partition), processed as
# K fused multiply-reduce ops of [128, HID] each.  Small first tiles so the
# DVE starts ASAP; small last tiles so the post-loop DVE tail is short.
K_MAX = 4
ENC_BUFS = 6
SCHEDULE = [1, 1, 2, 4, 4, 4, 4, 4, 4, 2, 1, 1]     # sums to N_COL = 32
assert sum(SCHEDULE) == N_COL


def build_body(nc, tc, enc, vb, ident, out, n_cores=N_CORES):
    f32 = mybir.dt.float32
    mx = mybir.AluOpType.max
    mult = mybir.AluOpType.mult
    groups = [list(range(n_cores))]

    ctx = ExitStack()
    cpool = ctx.enter_context(tc.tile_pool(name="cpool", bufs=1))
    iopool = ctx.enter_context(tc.tile_pool(name="iopool", bufs=ENC_BUFS))
    wpool = ctx.enter_context(tc.tile_pool(name="wpool", bufs=2))
    dwpool = ctx.enter_context(tc.tile_pool(name="dwpool", bufs=1, space="DRAM"))
    dgpool = ctx.enter_context(tc.tile_pool(name="dgpool", bufs=1, space="DRAM"))
    pspool = ctx.enter_context(tc.tile_pool(name="pspool", bufs=1, space="PSUM"))

    # Warm-up collective FIRST: its input is an UNWRITTEN internal DRAM tile
    # (the gathered garbage is never read), so the doorbell has no producer
    # dependency and fires as soon as the framework preamble ends; the ncfw
    # first-collective barrier (~54us of CC-core time, measured) then runs
    # concurrently with the main loop instead of in front of the real
    # (m, s) gather.
    warm_in = dwpool.tile([8], f32)
    warm_out = dwpool.tile([8 * n_cores], f32, addr_space="Shared")
    nc.gpsimd.collective_compute(
        "AllGather",
        mybir.AluOpType.bypass,
        replica_groups=groups,
        ins=[warm_in.opt()],
        outs=[warm_out.opt()],
    )

    # v on the scalar HWDGE ring so it doesn't queue behind enc tiles on the
    # sync ring (the two rings issue and drain concurrently).
    v_sb = cpool.tile([128, HID], f32)
    nc.scalar.dma_start(out=v_sb[:, :], in_=vb[:, :])

    # ones matrix generated on-chip (gpsimd is idle early); used for the
    # cross-partition sum and scalar broadcasts in the tail.
    ones_sb = cpool.tile([128, 128], f32)
    nc.gpsimd.memset(ones_sb[:, :], 1.0)

    # Early throwaway exp so the ~2.4us ACT_TABLE_LOAD(+drain) runs during
    # the main loop; without it the table load lands on the softmax
    # critical path right before the real exp.
    warm = wpool.tile([1, 1], f32, tag="warm")
    nc.scalar.activation(
        out=warm[:, :], in_=v_sb[0:1, 0:1],
        func=mybir.ActivationFunctionType.Exp,
        bias=v_sb[0:1, 0:1],
    )

    # --- main loop: e_sb[p, j] = energy of shard-local seq = p*N_COL + j ---
    # Partition-major layout: each partition's DMA line is one contiguous
    # kt*4KiB chunk of HBM, and the final attn store is contiguous too.
    e_sb = cpool.tile([128, N_COL], f32)
    enc_r = enc.rearrange("(p j) h -> p j h", p=128)   # [128, N_COL, HID]

    # ident is only needed in the tail; it rides the scalar ring behind v so
    # it never competes with the enc stream on the sync ring.
    ident_sb = cpool.tile([128, 128], f32)
    nc.scalar.dma_start(out=ident_sb[:, :], in_=ident[:, :])

    # All enc tiles go on the sync HWDGE ring, in consumption order: the
    # per-ring FIFO then delivers tiles in exactly the order the DVE needs
    # them (splitting them across rings makes early tiles complete late,
    # because in-flight transfers share SDMA bandwidth round-robin).
    j0 = 0
    for t, kt in enumerate(SCHEDULE):
        buf = iopool.tile([128, K_MAX * HID], f32, tag="enc")
        bufv = buf.rearrange("p (k h) -> p k h", k=K_MAX)
        nc.sync.dma_start(out=bufv[:, 0:kt, :], in_=enc_r[:, j0:j0 + kt, :])
        scratch = wpool.tile([128, HID], f32, tag="scratch")
        for k in range(kt):
            j = j0 + k
            # fused multiply + free-dim-sum: out = (in0 * 1.0) * v,
            # accum_out = sum(out).  (tensor_tensor_reduce crashes trn2 HW
            # under this compile path; scalar_tensor_tensor is equivalent.)
            nc.vector.scalar_tensor_tensor(
                out=scratch[:, :],
                in0=buf[:, k * HID:(k + 1) * HID],
                scalar=1.0,
                in1=v_sb[:, :],
                op0=mult,
                op1=mult,
                accum_out=e_sb[:, j:j + 1],
            )
        j0 += kt

    # --- tail: local softmax with local max ---
    m1 = wpool.tile([128, 1], f32, tag="m1", bufs=1)
    nc.vector.tensor_reduce(
        out=m1[:, :], in_=e_sb[:, :], axis=mybir.AxisListType.X, op=mx,
    )
    m1t_ps = pspool.tile([1, 128], f32, tag="m1t")
    nc.tensor.transpose(m1t_ps[:, :], m1[:, :], ident_sb[:, :])
    gmx = wpool.tile([1, 1], f32, tag="gmx", bufs=1)
    nc.vector.tensor_reduce(
        out=gmx[:, :], in_=m1t_ps[:, :], axis=mybir.AxisListType.X, op=mx,
    )
    ngm1 = wpool.tile([1, 1], f32, tag="ngm1", bufs=1)
    nc.vector.tensor_scalar_mul(ngm1[:, :], gmx[:, :], -1.0)
    nm_ps = pspool.tile([128, 1], f32, tag="nm")
    nc.tensor.matmul(nm_ps[:, :], ones_sb[0:1, :], ngm1[0:1, 0:1],
                     start=True, stop=True)
    nm_sb = wpool.tile([128, 1], f32, tag="nm_sb", bufs=1)
    nc.vector.tensor_copy(nm_sb[:, :], nm_ps[:, :])

    a_loc = cpool.tile([128, N_COL], f32)
    ssum = wpool.tile([128, 1], f32, tag="ssum", bufs=1)
    nc.scalar.activation(
        out=a_loc[:, :], in_=e_sb[:, :],
        func=mybir.ActivationFunctionType.Exp,
        bias=nm_sb[:, :], scale=1.0,
        accum_out=ssum[:, :],
    )
    s_ps = pspool.tile([1, 1], f32, tag="s")
    nc.tensor.matmul(s_ps[:, :], ssum[:, :], ones_sb[:, 0:1],
                     start=True, stop=True)

    # --- gather (m_i, s_i) from all cores: 8 bytes per core ---
    pk = wpool.tile([1, 2], f32, tag="pk", bufs=1)
    nc.vector.tensor_copy(pk[0:1, 0:1], gmx[0:1, 0:1])
    nc.vector.tensor_copy(pk[0:1, 1:2], s_ps[0:1, 0:1])
    ga_in = dgpool.tile([2], f32)
    ga_out = dgpool.tile([2 * n_cores], f32, addr_space="Shared")
    nc.sync.dma_start(out=ga_in.rearrange("(a b) -> a b", a=1),
                      in_=pk[0:1, 0:2])
    nc.gpsimd.collective_compute(
        "AllGather",
        mybir.AluOpType.bypass,
        replica_groups=groups,
        ins=[ga_in.opt()],
        outs=[ga_out.opt()],
    )
    g_r = ga_out.rearrange("(c k) -> k c", k=2)        # row 0: m's, row 1: s's
    mrow = wpool.tile([1, 8], f32, tag="mrow", bufs=1)
    srow = wpool.tile([1, 8], f32, tag="srow", bufs=1)
    nc.sync.dma_start(out=mrow[0:1, :], in_=g_r[0:1, :])
    nc.scalar.dma_start(out=srow[0:1, :], in_=g_r[1:2, :])

    # M = max_j m_j;  S = sum_j s_j * exp(m_j - M);  c = exp(m_i - M) / S
    M = wpool.tile([1, 1], f32, tag="M", bufs=1)
    nc.vector.tensor_reduce(
        out=M[:, :], in_=mrow[:, :], axis=mybir.AxisListType.X, op=mx,
    )
    nM = wpool.tile([1, 1], f32, tag="nM", bufs=1)
    nc.vector.tensor_scalar_mul(nM[:, :], M[:, :], -1.0)
    trow = wpool.tile([1, 8], f32, tag="trow", bufs=1)
    nc.scalar.activation(
        out=trow[:, :], in_=mrow[:, :],
        func=mybir.ActivationFunctionType.Exp,
        bias=nM[0:1, 0:1],
    )
    junk = wpool.tile([1, 8], f32, tag="junk", bufs=1)
    S = wpool.tile([1, 1], f32, tag="S", bufs=1)
    nc.vector.scalar_tensor_tensor(
        out=junk[:, :], in0=trow[:, :], scalar=1.0, in1=srow[:, :],
        op0=mult, op1=mult, accum_out=S[:, :],
    )
    rS = wpool.tile([1, 1], f32, tag="rS", bufs=1)
    nc.vector.reciprocal(rS[:, :], S[:, :])
    ee = wpool.tile([1, 1], f32, tag="ee", bufs=1)
    nc.scalar.activation(
        out=ee[:, :], in_=gmx[:, :],
        func=mybir.ActivationFunctionType.Exp,
        bias=nM[0:1, 0:1],
    )
    cfac = wpool.tile([1, 1], f32, tag="cfac", bufs=1)
    nc.vector.tensor_mul(cfac[:, :], ee[:, :], rS[:, :])
    c_ps = pspool.tile([128, 1], f32, tag="c")
    nc.tensor.matmul(c_ps[:, :], ones_sb[0:1, :], cfac[0:1, 0:1],
                     start=True, stop=True)
    c_sb = wpool.tile([128, 1], f32, tag="c_sb", bufs=1)
    nc.vector.tensor_copy(c_sb[:, :], c_ps[:, :])

    attn = cpool.tile([128, N_COL], f32)
    nc.vector.tensor_scalar_mul(attn[:, :], a_loc[:, :], c_sb[:, :])
    nc.sync.dma_start(out=out.rearrange("(p j) -> p j", p=128),
                      in_=attn[:, :])

    ctx.close()


def build_nc(n_cores=N_CORES, debug=False):
    nc = bacc.Bacc(
        "TRN2",
        target_bir_lowering=False,
        debug=debug,
        num_devices=n_cores,
    )
    enc = nc.dram_tensor("enc", [SHARD, HID], mybir.dt.float32, kind="ExternalInput")
    vb = nc.dram_tensor("vb", [128, HID], mybir.dt.float32, kind="ExternalInput")
    ident = nc.dram_tensor("ident", [128, 128], mybir.dt.float32, kind="ExternalInput")
    out = nc.dram_tensor("attn", [SHARD], mybir.dt.float32, kind="ExternalOutput")
    with tile.TileContext(nc) as tc:
        build_body(nc, tc, enc.ap(), vb.ap(), ident.ap(), out.ap(),
                   n_cores=n_cores)
    nc.compile()
    return nc


_NC_CACHE = {}


def _get_nc():
    if "nc" not in _NC_CACHE:
        _NC_CACHE["nc"] = build_nc()
    return _NC_CACHE["nc"]


def make_in_maps(hidden, encoder_outputs, attn_w, attn_b=None, n_cores=N_CORES,
                 shard=SHARD):
    hidden = np.asarray(hidden, dtype=np.float32)
    enc = np.asarray(encoder_outputs, dtype=np.float32)[0]
    w = np.asarray(attn_w, dtype=np.float32)
    v = (w.T @ hidden).astype(np.float32)
    vb = np.ascontiguousarray(np.broadcast_to(v[None, :], (128, v.shape[0])))
    ident = np.eye(128, dtype=np.float32)
    return [
        {
            "enc": np.ascontiguousarray(enc[i * shard:(i + 1) * shard, :]),
            "vb": vb,
            "ident": ident,
        }
        for i in range(n_cores)
    ]


def run(in_maps, trace=False, **kwargs):
    nc = _get_nc()
    return run_bass_kernel_spmd(
        nc, in_maps, core_ids=list(range(N_CORES)), trace=trace, **kwargs
    )


def kernel(**inputs):
    in_maps = make_in_maps(
        inputs["hidden"], inputs["encoder_outputs"], inputs["attn_w"],
        inputs.get("attn_b"),
    )
    res = run(in_maps)
    shards = [
        np.asarray(res.results[i]["attn"], dtype=np.float32).reshape(-1)
        for i in range(N_CORES)
    ]
    attn = np.concatenate(shards)
    return attn[None, None, :]


# revision 12
# speedup vs baseline: 1.5029x; 1.0438x over previous
"""Collective-free device kernel; softmax shards merged on host.

Device (per core): energies e[p,j] for its seq shard (partition-major:
seq = p*32 + j), per-PARTITION max m[p], a[p,j] = exp(e[p,j] - m[p]) and
s[p] = sum_j a[p,j].  Outputs a [4096] and packed (m, s) [256].
Host: M = max over all 1024 m's, S = sum s*exp(m-M), then scales each
partition row by exp(m-M)/S while unsharding.  Using the per-partition max
as the local stabilizer is exact (log-sum-exp merge) and removes every
cross-partition reduction from the device tail.
"""

import sys

sys.path.insert(0, "/opt/trn_rl_repo")

from contextlib import ExitStack

import numpy as np

import concourse.bacc as bacc
import concourse.mybir as mybir
import concourse.tile as tile
from concourse.bass_utils import run_bass_kernel_spmd

N_CORES = 8
SEQ = 32768
HID = 1024
SHARD = SEQ // N_CORES      # 4096
N_COL = SHARD // 128        # 32

K_MAX = 8
ENC_BUFS = 4
SCHEDULE = [1, 1, 2, 4, 8, 8, 4, 2, 1, 1]
assert sum(SCHEDULE) == N_COL


def build_body(nc, tc, enc, vb, out, ms_out):
    f32 = mybir.dt.float32
    mx = mybir.AluOpType.max
    mult = mybir.AluOpType.mult

    ctx = ExitStack()
    cpool = ctx.enter_context(tc.tile_pool(name="cpool", bufs=1))
    iopool = ctx.enter_context(tc.tile_pool(name="iopool", bufs=ENC_BUFS))
    wpool = ctx.enter_context(tc.tile_pool(name="wpool", bufs=2))

    # v arrives as a 4KB row and is broadcast on-chip: saves 0.5MB of HBM
    # that would compete with the enc stream.  The SWDGE broadcast latency
    # (~6us) delays the first multiply-reduce, but the loop is DMA-bound so
    # the DVE has slack to absorb it (A/B-measured faster than a direct
    # [128, HID] v load).
    v0 = cpool.tile([1, HID], f32)
    nc.scalar.dma_start(out=v0[0:1, :], in_=vb[:, :])
    v_sb = cpool.tile([128, HID], f32)
    nc.gpsimd.partition_broadcast(v_sb[:, :], v0[0:1, :])

    # Early throwaway exp so the ACT_TABLE_LOAD runs during the main loop,
    # not in front of the tail exp.
    warm = wpool.tile([1, 1], f32, tag="warm")
    nc.scalar.activation(
        out=warm[:, :], in_=v0[0:1, 0:1],
        func=mybir.ActivationFunctionType.Exp,
        bias=v0[0:1, 0:1],
    )

    # --- main loop: e_sb[p, j] = energy of shard-local seq = p*N_COL + j ---
    e_sb = cpool.tile([128, N_COL], f32)
    enc_r = enc.rearrange("(p j) h -> p j h", p=128)

    j0 = 0
    for t, kt in enumerate(SCHEDULE):
        buf = iopool.tile([128, K_MAX * HID], f32, tag="enc")
        bufv = buf.rearrange("p (k h) -> p k h", k=K_MAX)
        nc.sync.dma_start(out=bufv[:, 0:kt, :], in_=enc_r[:, j0:j0 + kt, :])
        scratch = wpool.tile([128, HID], f32, tag="scratch")
        for k in range(kt):
            j = j0 + k
            # fused multiply + free-dim-sum: out = (in0 * 1.0) * v,
            # accum_out = sum(out).  (tensor_tensor_reduce crashes trn2 HW
            # under this compile path; scalar_tensor_tensor is equivalent.)
            nc.vector.scalar_tensor_tensor(
                out=scratch[:, :],
                in0=buf[:, k * HID:(k + 1) * HID],
                scalar=1.0,
                in1=v_sb[:, :],
                op0=mult,
                op1=mult,
                accum_out=e_sb[:, j:j + 1],
            )
        j0 += kt

    # --- tail: per-partition softmax pieces, no cross-partition reduction ---
    m1 = wpool.tile([128, 1], f32, tag="m1", bufs=1)
    nc.vector.tensor_reduce(
        out=m1[:, :], in_=e_sb[:, :], axis=mybir.AxisListType.X, op=mx,
    )
    nm1 = wpool.tile([128, 1], f32, tag="nm1", bufs=1)
    nc.vector.tensor_scalar_mul(nm1[:, :], m1[:, :], -1.0)

    a_loc = cpool.tile([128, N_COL], f32)
    ssum = wpool.tile([128, 1], f32, tag="ssum", bufs=1)
    nc.scalar.activation(
        out=a_loc[:, :], in_=e_sb[:, :],
        func=mybir.ActivationFunctionType.Exp,
        bias=nm1[:, :], scale=1.0,
        accum_out=ssum[:, :],
    )

    pk = wpool.tile([128, 2], f32, tag="pk", bufs=1)
    nc.vector.tensor_copy(pk[:, 0:1], m1[:, :])
    nc.vector.tensor_copy(pk[:, 1:2], ssum[:, :])

    nc.sync.dma_start(out=out.rearrange("(p j) -> p j", p=128),
                      in_=a_loc[:, :])
    nc.scalar.dma_start(out=ms_out.rearrange("(p k) -> p k", k=2),
                        in_=pk[:, :])

    ctx.close()


def build_nc(n_cores=N_CORES, debug=False):
    nc = bacc.Bacc(
        "TRN2",
        target_bir_lowering=False,
        debug=debug,
        num_devices=n_cores,
    )
    enc = nc.dram_tensor("enc", [SHARD, HID], mybir.dt.float32, kind="ExternalInput")
    vb = nc.dram_tensor("vb", [1, HID], mybir.dt.float32, kind="ExternalInput")
    out = nc.dram_tensor("attn_part", [SHARD], mybir.dt.float32,
                         kind="ExternalOutput")
    ms = nc.dram_tensor("ms", [2 * 128], mybir.dt.float32, kind="ExternalOutput")
    with tile.TileContext(nc) as tc:
        build_body(nc, tc, enc.ap(), vb.ap(), out.ap(), ms.ap())
    nc.compile()
    return nc


_NC_CACHE = {}


def _get_nc():
    if "nc" not in _NC_CACHE:
        _NC_CACHE["nc"] = build_nc()
    return _NC_CACHE["nc"]


def make_in_maps(hidden, encoder_outputs, attn_w, attn_b=None, n_cores=N_CORES,
                 shard=SHARD):
    hidden = np.asarray(hidden, dtype=np.float32)
    enc = np.asarray(encoder_outputs, dtype=np.float32)[0]
    w = np.asarray(attn_w, dtype=np.float32)
    v = (w.T @ hidden).astype(np.float32)
    vb = np.ascontiguousarray(v[None, :])
    return [
        {
            "enc": np.ascontiguousarray(enc[i * shard:(i + 1) * shard, :]),
            "vb": vb,
        }
        for i in range(n_cores)
    ]


def run(in_maps, trace=False, **kwargs):
    nc = _get_nc()
    return run_bass_kernel_spmd(
        nc, in_maps, core_ids=list(range(N_CORES)), trace=trace, **kwargs
    )


def kernel(**inputs):
    in_maps = make_in_maps(
        inputs["hidden"], inputs["encoder_outputs"], inputs["attn_w"],
        inputs.get("attn_b"),
    )
    res = run(in_maps)
    parts = [
        np.asarray(res.results[i]["attn_part"], dtype=np.float32).reshape(128, N_COL)
        for i in range(N_CORES)
    ]
    ms = [
        np.asarray(res.results[i]["ms"], dtype=np.float32).reshape(128, 2)
        for i in range(N_CORES)
    ]
    m = np.stack([x[:, 0] for x in ms]).astype(np.float64)   # [8, 128]
    s = np.stack([x[:, 1] for x in ms]).astype(np.float64)   # [8, 128]
    M = m.max()
    w = np.exp(m - M)                                        # [8, 128]
    S = float((s * w).sum())
    scale = (w / S).astype(np.float32)                       # [8, 128]
    attn = np.concatenate(
        [(parts[i] * scale[i][:, None]).reshape(-1) for i in range(N_CORES)]
    )
    return attn[None, None, :]


# revision 17
# speedup vs baseline: 1.6894x; 1.1241x over previous
"""Collective-free device kernel; softmax shards merged on host.

Device (per core): energies e[p,j] for its seq shard (partition-major:
seq = p*32 + j), per-PARTITION max m[p], a[p,j] = exp(e[p,j] - m[p]) and
s[p] = sum_j a[p,j].  Outputs a [4096] and packed (m, s) [256].
Host: M = max over all 1024 m's, S = sum s*exp(m-M), then scales each
partition row by exp(m-M)/S while unsharding.  Using the per-partition max
as the local stabilizer is exact (log-sum-exp merge) and removes every
cross-partition reduction from the device tail.
"""

import sys

sys.path.insert(0, "/opt/trn_rl_repo")

from contextlib import ExitStack

import numpy as np

import concourse.bacc as bacc
import concourse.mybir as mybir
import concourse.tile as tile
from concourse.bass_utils import run_bass_kernel_spmd

N_CORES = 8
SEQ = 32768
HID = 1024
SHARD = SEQ // N_CORES      # 4096
N_COL = SHARD // 128        # 32

K_MAX = 8
ENC_BUFS = 4
SCHEDULE = [1, 1, 2, 4, 8, 8, 4, 2, 1, 1]
assert sum(SCHEDULE) == N_COL


def build_body(nc, tc, enc, vb, out, ms_out):
    f32 = mybir.dt.float32
    mx = mybir.AluOpType.max
    mult = mybir.AluOpType.mult

    ctx = ExitStack()
    cpool = ctx.enter_context(tc.tile_pool(name="cpool", bufs=1))
    iopool = ctx.enter_context(tc.tile_pool(name="iopool", bufs=ENC_BUFS))
    wpool = ctx.enter_context(tc.tile_pool(name="wpool", bufs=2))

    f16 = mybir.dt.float16

    # v pre-broadcast on host in fp16 (256KB), on the scalar HWDGE ring so
    # it doesn't queue behind the enc tiles.
    v_sb = cpool.tile([128, HID], f16)
    nc.scalar.dma_start(out=v_sb[:, :], in_=vb[:, :])

    # Early throwaway exp so the ACT_TABLE_LOAD runs during the main loop,
    # not in front of the tail exp.
    warm = wpool.tile([1, 1], f32, tag="warm")
    nc.scalar.activation(
        out=warm[:, :], in_=v_sb[0:1, 0:1],
        func=mybir.ActivationFunctionType.Exp,
    )

    # --- main loop: e_sb[p, j] = energy of shard-local seq = p*N_COL + j ---
    e_sb = cpool.tile([128, N_COL], f32)
    enc_r = enc.rearrange("(p j) h -> p j h", p=128)

    j0 = 0
    for t, kt in enumerate(SCHEDULE):
        buf = iopool.tile([128, K_MAX * HID], f16, tag="enc")
        bufv = buf.rearrange("p (k h) -> p k h", k=K_MAX)
        nc.sync.dma_start(out=bufv[:, 0:kt, :], in_=enc_r[:, j0:j0 + kt, :])
        scratch = wpool.tile([128, HID], f16, tag="scratch")
        for k in range(kt):
            j = j0 + k
            # fused multiply + free-dim-sum: out = (in0 * 1.0) * v,
            # accum_out = sum(out).  (tensor_tensor_reduce crashes trn2 HW
            # under this compile path; scalar_tensor_tensor is equivalent.)
            nc.vector.scalar_tensor_tensor(
                out=scratch[:, :],
                in0=buf[:, k * HID:(k + 1) * HID],
                scalar=1.0,
                in1=v_sb[:, :],
                op0=mult,
                op1=mult,
                accum_out=e_sb[:, j:j + 1],
            )
        j0 += kt

    # --- tail: per-partition softmax pieces, no cross-partition reduction ---
    m1 = wpool.tile([128, 1], f32, tag="m1", bufs=1)
    nc.vector.tensor_reduce(
        out=m1[:, :], in_=e_sb[:, :], axis=mybir.AxisListType.X, op=mx,
    )
    nm1 = wpool.tile([128, 1], f32, tag="nm1", bufs=1)
    nc.vector.tensor_scalar_mul(nm1[:, :], m1[:, :], -1.0)

    a_loc = cpool.tile([128, N_COL], f32)
    ssum = wpool.tile([128, 1], f32, tag="ssum", bufs=1)
    nc.scalar.activation(
        out=a_loc[:, :], in_=e_sb[:, :],
        func=mybir.ActivationFunctionType.Exp,
        bias=nm1[:, :], scale=1.0,
        accum_out=ssum[:, :],
    )

    pk = wpool.tile([128, 2], f32, tag="pk", bufs=1)
    nc.vector.tensor_copy(pk[:, 0:1], m1[:, :])
    nc.vector.tensor_copy(pk[:, 1:2], ssum[:, :])

    nc.sync.dma_start(out=out.rearrange("(p j) -> p j", p=128),
                      in_=a_loc[:, :])
    nc.scalar.dma_start(out=ms_out.rearrange("(p k) -> p k", k=2),
                        in_=pk[:, :])

    ctx.close()


def build_nc(n_cores=N_CORES, debug=False):
    nc = bacc.Bacc(
        "TRN2",
        target_bir_lowering=False,
        debug=debug,
        num_devices=n_cores,
    )
    enc = nc.dram_tensor("enc", [SHARD, HID], mybir.dt.float16, kind="ExternalInput")
    vb = nc.dram_tensor("vb", [128, HID], mybir.dt.float16, kind="ExternalInput")
    out = nc.dram_tensor("attn_part", [SHARD], mybir.dt.float32,
                         kind="ExternalOutput")
    ms = nc.dram_tensor("ms", [2 * 128], mybir.dt.float32, kind="ExternalOutput")
    with tile.TileContext(nc) as tc:
        build_body(nc, tc, enc.ap(), vb.ap(), out.ap(), ms.ap())
    nc.compile()
    return nc


_NC_CACHE = {}


def _get_nc():
    if "nc" not in _NC_CACHE:
        _NC_CACHE["nc"] = build_nc()
    return _NC_CACHE["nc"]


def make_in_maps(hidden, encoder_outputs, attn_w, attn_b=None, n_cores=N_CORES,
                 shard=SHARD):
    hidden = np.asarray(hidden, dtype=np.float32)
    enc = np.asarray(encoder_outputs, dtype=np.float32)[0]
    w = np.asarray(attn_w, dtype=np.float32)
    v = (w.T @ hidden).astype(np.float32)
    # fp16 streaming: halves the HBM traffic of the enc stream and enables
    # the DVE 16-bit 2x mode.  Softmax rel err vs the f32 reference is
    # ~3.2e-3 (quantization of enc and v only; products and accumulation
    # stay fp32 on device).
    vb = np.ascontiguousarray(
        np.broadcast_to(v.astype(np.float16)[None, :], (128, v.shape[0])))
    return [
        {
            "enc": np.ascontiguousarray(
                enc[i * shard:(i + 1) * shard, :].astype(np.float16)),
            "vb": vb,
        }
        for i in range(n_cores)
    ]


def run(in_maps, trace=False, **kwargs):
    nc = _get_nc()
    return run_bass_kernel_spmd(
        nc, in_maps, core_ids=list(range(N_CORES)), trace=trace, **kwargs
    )


def kernel(**inputs):
    in_maps = make_in_maps(
        inputs["hidden"], inputs["encoder_outputs"], inputs["attn_w"],
        inputs.get("attn_b"),
    )
    res = run(in_maps)
    parts = [
        np.asarray(res.results[i]["attn_part"], dtype=np.float32).reshape(128, N_COL)
        for i in range(N_CORES)
    ]
    ms = [
        np.asarray(res.results[i]["ms"], dtype=np.float32).reshape(128, 2)
        for i in range(N_CORES)
    ]
    m = np.stack([x[:, 0] for x in ms]).astype(np.float64)   # [8, 128]
    s = np.stack([x[:, 1] for x in ms]).astype(np.float64)   # [8, 128]
    M = m.max()
    w = np.exp(m - M)                                        # [8, 128]
    S = float((s * w).sum())
    scale = (w / S).astype(np.float32)                       # [8, 128]
    attn = np.concatenate(
        [(parts[i] * scale[i][:, None]).reshape(-1) for i in range(N_CORES)]
    )
    return attn[None, None, :]
